# revision 1
# baseline (speedup 1.0000x reference)
"""Sliding-window GQA attention on 8 trn2 NeuronCores.

Sharding: 8 cores = 2 batches x 4 KV groups. Core c=(4*b+g) handles batch b
and query heads [4g, 4g+4) (which share kv head g). Each core computes a
partial output x_b-attention-wo_g; the host sums the 4 group partials per
batch (the wo contraction is split across groups).

All heavy matmuls run as float32r (TF32-like, full PE rate at N>=256,
~1e-3 relative accuracy); the P@V stage runs bf16 (P in [0,1]).
"""
import os
import sys

sys.path.insert(0, "/opt/trn_rl_repo")

import numpy as np

import concourse.bass as bass
import concourse.tile as tile
from concourse import bacc, mybir
from concourse.bass_utils import run_bass_kernel_spmd

B, L, DIM = 2, 2048, 2048
NH, NKV, HD = 16, 4, 128
W = 512
NHL = 4          # query heads per core
GDIM = NHL * HD  # 512 head-dims per core
SCALE = float(HD) ** -0.5
MASKVAL = -60000.0
TOKC = 256       # token chunk for projection matmuls (fp32r full-rate needs N>=256)
NTC = L // TOKC
KC = DIM // 128  # contraction chunks for projections
NB = L // 128    # query blocks
F32 = mybir.dt.float32
F32R = mybir.dt.float32r
BF16 = mybir.dt.bfloat16

_built = {}
last_results = None


def _key_range(i):
    """Keys needed by query block i: [klo, klo+wk)."""
    if i < 4:
        return 0, 128 * (i + 1)
    return 128 * (i - 4), 640


def _s_chunks(wk):
    """Split S width into matmul chunks that stay inside one PSUM bank."""
    if wk <= 512:
        return [(0, wk)]
    return [(0, 512), (512, wk - 512)]


def _build(reps=1, phases="ABC"):
    key = (reps, phases)
    if key in _built:
        return _built[key]

    nc = bacc.Bacc("TRN2", target_bir_lowering=False, debug=False,
                   enable_asserts=False)
    xT = nc.dram_tensor("xT", [DIM, L], F32R, kind="ExternalInput").ap()
    wq = nc.dram_tensor("wq", [DIM, GDIM], F32R, kind="ExternalInput").ap()
    wk = nc.dram_tensor("wk", [DIM, HD], F32R, kind="ExternalInput").ap()
    wv = nc.dram_tensor("wv", [DIM, HD], F32R, kind="ExternalInput").ap()
    wo = nc.dram_tensor("wo", [GDIM, DIM], F32R, kind="ExternalInput").ap()
    out = nc.dram_tensor("out", [L, DIM], F32, kind="ExternalOutput").ap()

    TOK = 512                 # token chunk (N of projection matmuls)
    NT = L // TOK             # 4 chunks
    KH = KC // 2              # 8 contraction chunks per half slab

    with tile.TileContext(nc) as tc:
      with tc.tile_pool(name="persist", bufs=1) as pers:
        # --- constants ---
        ident32 = pers.tile([128, 128], F32, tag="ident32")
        ident16 = pers.tile([128, 128], BF16, tag="ident16")
        for ident in (ident32, ident16):
            nc.gpsimd.memset(ident, 0.0)
            nc.gpsimd.affine_select(
                out=ident, in_=ident, compare_op=mybir.AluOpType.not_equal,
                fill=1.0, base=0, pattern=[[-1, 128]], channel_multiplier=1)
        ident_r = pers.tile([128, 128], F32R, tag="ident_r")
        nc.vector.tensor_copy(ident_r, ident32)
        # additive triangle masks, bf16 (applied via PE matmul pre-write):
        # mask[:,0,:] upper-incl valid (c>=r), mask[:,1,:] causal valid (c<=r)
        mask = pers.tile([128, 2, 128], BF16, tag="mask")
        up, lo = mask[:, 0, :], mask[:, 1, :]
        nc.gpsimd.memset(up, 0.0)
        nc.gpsimd.affine_select(
            out=up, in_=up, compare_op=mybir.AluOpType.is_ge,
            fill=MASKVAL, base=0, pattern=[[1, 128]], channel_multiplier=-1)
        nc.gpsimd.memset(lo, 0.0)
        nc.gpsimd.affine_select(
            out=lo, in_=lo, compare_op=mybir.AluOpType.is_ge,
            fill=MASKVAL, base=0, pattern=[[-1, 128]], channel_multiplier=1)

        # --- persistent tensors ---
        qT = [pers.tile([128, L], F32R, tag=f"qT{m}", name=f"qT{m}")
              for m in range(NHL)]
        kT = pers.tile([128, L], F32R, tag="kT")
        vnat = pers.tile([128, KC, 128], F32R, tag="vnat")
        # oT_all[:, i, h, :] = O^T[d, q] for head h, query block i
        oT_all = pers.tile([128, NB, NHL, 128], F32R, tag="oT_all")

        def mask_mm(dst, which):
            """Pre-write a triangle mask into PSUM via PE (sets has_written;
            the S matmul then accumulates on top with start=False)."""
            nc.tensor.matmul(dst, ident16, mask[:, which, :],
                             start=True, stop=False, skip_group_check=True)

        for _rep in range(reps):
            # ================= Phase A: projections =================
            with tc.tile_pool(name="paw", bufs=1) as paw, \
                 tc.tile_pool(name="pax", bufs=3) as pax, \
                 tc.tile_pool(name="psA", bufs=4, space="PSUM") as psA, \
                 tc.tile_pool(name="psV", bufs=2, space="PSUM") as psV:
                wq_sb = paw.tile([128, KC, GDIM], F32R, tag="wq")
                wk_sb = paw.tile([128, KC, HD], F32R, tag="wk")
                wv_sb = paw.tile([128, KC, HD], F32R, tag="wv")
                wq_r = wq.rearrange("(kc p) n -> p kc n", p=128)
                wk_r = wk.rearrange("(kc p) n -> p kc n", p=128)
                wv_r = wv.rearrange("(kc p) n -> p kc n", p=128)
                xT_r = xT.rearrange("(kc p) t -> p kc t", p=128)
                vT = paw.tile([128, L], F32, tag="vT")

                def load_half(n, half):
                    # two quarter-slab DMAs into one half-slab tile so the
                    # first matmuls unblock after a quarter of the data
                    xh = pax.tile([128, KH, TOK], F32R, tag="x", name="xh")
                    for q in range(2):
                        ks = np.s_[:, half * KH + 4 * q:half * KH + 4 * (q + 1),
                                   n * TOK:(n + 1) * TOK]
                        nc.sync.dma_start(out=xh[:, 4 * q:4 * (q + 1), :],
                                          in_=xT_r[ks])
                    return xh

                for n in range(NT):
                    if n == 0:
                        x_lo = load_half(0, 0)
                        # weights: kc quarters, q first (m-loop starts with q)
                        for q4 in range(4):
                            ksl = np.s_[:, 4 * q4:4 * (q4 + 1), :]
                            nc.sync.dma_start(out=wq_sb[ksl], in_=wq_r[ksl])
                            nc.sync.dma_start(out=wk_sb[ksl], in_=wk_r[ksl])
                            nc.sync.dma_start(out=wv_sb[ksl], in_=wv_r[ksl])
                        x_hi = load_half(0, 1)
                    for m in range(NHL + 2):
                        acc = psA.tile([128, TOK], F32, tag="acc")
                        for kc in range(KC):
                            if m < NHL:
                                lhsT = wq_sb[:, kc, 128 * m:128 * (m + 1)]
                            elif m == NHL:
                                lhsT = wk_sb[:, kc, :]
                            else:
                                lhsT = wv_sb[:, kc, :]
                            xh = x_lo if kc < KH else x_hi
                            nc.tensor.matmul(acc, lhsT, xh[:, kc % KH, :],
                                             start=(kc == 0),
                                             stop=(kc == KC - 1))
                        sl = np.s_[:, n * TOK:(n + 1) * TOK]
                        if m < NHL:
                            nc.vector.tensor_copy(qT[m][sl], acc)
                        elif m == NHL:
                            nc.vector.tensor_copy(kT[sl], acc)
                        else:
                            nc.vector.tensor_copy(vT[sl], acc)
                        # prefetch next half-slab midway through the chunk
                        if m == 0 and n + 1 < NT:
                            nx_lo = load_half(n + 1, 0)
                        if m == 2 and n + 1 < NT:
                            nx_hi = load_half(n + 1, 1)
                    # V -> natural [key, d] layout for this chunk's keys
                    for t in range(4 * n, 4 * (n + 1)):
                        pv = psV.tile([128, 128], F32, tag="pv")
                        nc.tensor.transpose(pv, vT[:, 128 * t:128 * (t + 1)],
                                            ident32)
                        nc.vector.tensor_copy(vnat[:, t, :], pv)
                    if n + 1 < NT:
                        x_lo, x_hi = nx_lo, nx_hi

            # ========= Phases B+C interleaved: attention + wo =========
            # The 4 query heads share the KV head, so P@V batches heads
            # along the free dim (N=512 fp32r). Phase C tile tt only needs
            # attention block tt, so it is emitted right behind it.
            with tc.tile_pool(name="pbw", bufs=1) as pbw, \
                 tc.tile_pool(name="pb", bufs=2) as pb, \
                 tc.tile_pool(name="pco", bufs=4) as pco, \
                 tc.tile_pool(name="psS", bufs=2, space="PSUM") as psS, \
                 tc.tile_pool(name="psT", bufs=1, space="PSUM") as psT, \
                 tc.tile_pool(name="psO", bufs=1, space="PSUM") as psO, \
                 tc.tile_pool(name="psC", bufs=2, space="PSUM") as psC:
                wo_sb = pbw.tile([128, NHL, DIM], F32R, tag="wo")
                nc.sync.dma_start(out=wo_sb,
                                  in_=wo.rearrange("(kc p) n -> p kc n", p=128))

                p_saved = {}

                def attn_front(i):
                    klo, wkk = _key_range(i)
                    nch = wkk // 128
                    p_tiles = []
                    for h in range(NHL):
                        s_ps = psS.tile([128, 640], F32, tag="s", name="s_ps")
                        if i < 4:
                            # causal mask on the last 128 cols, then S on top
                            mask_mm(s_ps[:, wkk - 128:wkk], 1)
                            nc.tensor.matmul(
                                s_ps[:, 0:wkk],
                                qT[h][:, 128 * i:128 * (i + 1)],
                                kT[:, klo:klo + wkk],
                                start=False, stop=True, skip_group_check=True)
                        else:
                            mask_mm(s_ps[:, 0:128], 0)
                            nc.tensor.matmul(
                                s_ps[:, 0:512],
                                qT[h][:, 128 * i:128 * (i + 1)],
                                kT[:, klo:klo + 512],
                                start=False, stop=True, skip_group_check=True)
                            mask_mm(s_ps[:, 512:640], 1)
                            nc.tensor.matmul(
                                s_ps[:, 512:640],
                                qT[h][:, 128 * i:128 * (i + 1)],
                                kT[:, klo + 512:klo + 640],
                                start=False, stop=True, skip_group_check=True)
                        e_sb = pb.tile([128, 640], F32, tag=f"e{h}",
                                       name="e_sb", bufs=2)
                        lsum = pb.tile([128, 1], F32, tag=f"l{h}", name="lsum",
                                       bufs=2)
                        p_sb = pb.tile([128, 640], F32R, tag=f"p{h}",
                                       name="p_sb", bufs=2)
                        nc.scalar.activation(
                            out=e_sb[:, :wkk], in_=s_ps[:, :wkk],
                            func=mybir.ActivationFunctionType.Exp,
                            scale=SCALE, accum_out=lsum)
                        linv = pb.tile([128, 1], F32, tag=f"li{h}",
                                       name="linv", bufs=2)
                        nc.vector.reciprocal(linv, lsum)
                        # NB: DVE, not gpsimd — gpsimd tensor_scalar costs
                        # ~7us/op on HW and serializes the block pipeline
                        nc.vector.tensor_scalar_mul(p_sb[:, :wkk],
                                                    e_sb[:, :wkk], linv)
                        p_tiles.append(p_sb)
                    p_saved[i] = p_tiles

                def attn_back(i):
                    klo, wkk = _key_range(i)
                    nch = wkk // 128
                    p_tiles = p_saved.pop(i)
                    # transpose P chunks for all heads: PT[kc][k, 128h+q]
                    ptq = pb.tile([128, 5, 512], F32R, tag="ptq", name="ptq")
                    for c in range(nch):
                        t_ps = psT.tile([128, 512], F32R, tag="t", name="t_ps")
                        for h in range(NHL):
                            nc.tensor.transpose(
                                t_ps[:, 128 * h:128 * (h + 1)],
                                p_tiles[h][:, 128 * c:128 * (c + 1)], ident_r)
                        nc.vector.tensor_copy(ptq[:, c, :], t_ps)
                    # O^T accumulation over key chunks (all heads at once)
                    o_ps = psO.tile([128, 512], F32, tag="o", name="o_ps")
                    for c in range(nch):
                        nc.tensor.matmul(
                            o_ps, vnat[:, klo // 128 + c, :], ptq[:, c, :],
                            start=(c == 0), stop=(c == nch - 1))
                    nc.vector.tensor_copy(oT_all[:, i, :, :], o_ps)

                def out_tile(tt):
                    for nn in range(4):
                        acc = psC.tile([128, 512], F32, tag="acc", name="acc")
                        for kc in range(NHL):
                            nc.tensor.matmul(
                                acc, oT_all[:, tt, kc, :],
                                wo_sb[:, kc, 512 * nn:512 * (nn + 1)],
                                start=(kc == 0), stop=(kc == NHL - 1))
                        o_sb = pco.tile([128, 512], F32, tag="o", name="o_sb")
                        nc.vector.tensor_copy(o_sb, acc)
                        nc.sync.dma_start(
                            out=out[128 * tt:128 * (tt + 1),
                                    512 * nn:512 * (nn + 1)],
                            in_=o_sb)

                if "B" in phases:
                    for i in range(NB):
                        attn_front(i)
                        if i >= 1:
                            attn_back(i - 1)
                        if "C" in phases and i >= 2:
                            out_tile(i - 2)
                    attn_back(NB - 1)
                    if "C" in phases:
                        out_tile(NB - 2)
                        out_tile(NB - 1)

    nc.compile()
    _built[key] = nc
    return nc


def kernel(x, wq, wk, wv, wo):
    global last_results
    x = np.ascontiguousarray(np.asarray(x, dtype=np.float32))
    wq = np.ascontiguousarray(np.asarray(wq, dtype=np.float32))
    wk = np.ascontiguousarray(np.asarray(wk, dtype=np.float32))
    wv = np.ascontiguousarray(np.asarray(wv, dtype=np.float32))
    wo = np.ascontiguousarray(np.asarray(wo, dtype=np.float32))

    nc = _build()
    xT = [np.ascontiguousarray(x[b].T) for b in range(B)]
    in_maps = []
    for c in range(8):
        b, g = c // 4, c % 4
        in_maps.append({
            "xT": xT[b],
            "wq": np.ascontiguousarray(wq[:, GDIM * g:GDIM * (g + 1)]),
            "wk": np.ascontiguousarray(wk[:, HD * g:HD * (g + 1)]),
            "wv": np.ascontiguousarray(wv[:, HD * g:HD * (g + 1)]),
            "wo": np.ascontiguousarray(wo[GDIM * g:GDIM * (g + 1), :]),
        })
    res = run_bass_kernel_spmd(nc, in_maps, list(range(8)))
    last_results = res
    out = np.empty((B, L, DIM), dtype=np.float32)
    for b in range(B):
        acc = np.zeros((L, DIM), dtype=np.float64)
        for g in range(4):
            acc += res.results[4 * b + g]["out"]
        out[b] = acc.astype(np.float32)
    return out

